# revision 11
# baseline (speedup 1.0000x reference)
"""Trainium2 Bass kernel for nn_BertSelfAttention_43404939493966.

BERT self-attention with adaptive per-segment scaling:
  q/k/v = hidden @ W{q,k,v}.T + b        (biases are spec'd zero -> skipped)
  scores = q k^T / 8,  scaled per (batch,row,col) segment rule, softmax, @v

Sharding: 8 cores = 4 batches x 2 head-groups (8 heads each).
Each core gets host-pretransposed bf16 operands:
  xt  = hidden[b].T            [H=1024, S=1024]
  w?t = W[g*512:(g+1)*512].T   [1024, 512]
  wm1 = (w_seg(q) - 1)         [1, S]   (w_seg = w0c if q < idx2 else w1c)
  mkey= 1[key >= idx2]         [1, S]
and returns ctx^T for its head-group  [512, S] f32.

Device algorithm (per core, one SPMD program):
  Segment scaling is exact via scale(k,q) = 1 + mkey(k)*(w(q)-1):
    scoresT = KT^T.QT + (KT*mkey)^T.(QT*(w-1))
  Both terms are computed in a SINGLE full-width (K=128) matmul by
  stacking per head h the pair [k_h ; k_h*mkey] (kaug) against
  [q_h ; q_h*(w-1)] (qaug) on the partition axis.  The stacked halves
  are built from the projection psums with partition-aligned DVE
  copies, a partition-shifted SBUF->SBUF DMA duplicate, and an aligned
  DVE multiply (even heads: raw top/scaled bottom; odd heads reversed,
  matching the psum half each head lands in).
  exp on ScalarE (scale=1/8 folded into the activation), output bf16.
  ctx^T = V_aug^T @ probsT with V augmented by a ones-column, so the
  softmax denominator falls out of the same matmul (psum row 64);
  normalize with reciprocal + partition-broadcast + multiply.

attention_mask is all-zeros by spec (fill=zeros) and is not applied.
"""

import numpy as np
import ml_dtypes
from contextlib import ExitStack

import concourse.bass as bass
import concourse.tile as tile
from concourse import bacc, mybir
from concourse.bass_utils import run_bass_kernel_spmd

B, S, H = 4, 1024, 1024
NH, HD = 16, 64
NCORES = 8
HG = 512          # head-group width (8 heads x 64)
KC = 8            # 128-wide key chunks
PC = 128

BF16 = mybir.dt.bfloat16
F32 = mybir.dt.float32


def _build_program():
    nc = bacc.Bacc("TRN2", target_bir_lowering=False, debug=False)

    XT = nc.dram_tensor("xt", (H, S), BF16, kind="ExternalInput")
    WQT = nc.dram_tensor("wqt", (H, HG), BF16, kind="ExternalInput")
    WKT = nc.dram_tensor("wkt", (H, HG), BF16, kind="ExternalInput")
    WVT = nc.dram_tensor("wvt", (H, HG), BF16, kind="ExternalInput")
    WM1 = nc.dram_tensor("wm1", (1, S), BF16, kind="ExternalInput")
    MKEY = nc.dram_tensor("mkey", (1, S), BF16, kind="ExternalInput")
    OUT = nc.dram_tensor("out_t", (HG, S), F32, kind="ExternalOutput")

    Exp = mybir.ActivationFunctionType.Exp

    with tile.TileContext(nc) as tc:
        with ExitStack() as ctx:
            persist = ctx.enter_context(tc.tile_pool(name="persist", bufs=1))

            # stacked score operands: [:, h, :] is head h's 128-deep
            # contraction tile ([raw;scaled] even h, [scaled;raw] odd h)
            qaug = persist.tile([PC, 8, S], BF16)
            kaug = persist.tile([PC, 8, S], BF16)
            vaug = persist.tile([PC, 8, 8, HD + 1], BF16)  # [p, s-chunk, head, d+1]
            wm1b = persist.tile([PC, S], BF16)
            mkb = persist.tile([PC, S], BF16)

            # load the per-q / per-key vectors ([1,S] rows), broadcast on
            # GpSimd (keeps the startup DMA path free for the big loads)
            wrow = persist.tile([1, S], BF16)
            mrow = persist.tile([1, S], BF16)
            nc.sync.dma_start(wrow, WM1[:, :])
            nc.sync.dma_start(mrow, MKEY[:, :])
            nc.gpsimd.partition_broadcast(wm1b, wrow)
            nc.gpsimd.partition_broadcast(mkb, mrow)
            nc.vector.memset(vaug[:, :, :, HD:HD + 1], 1.0)

            # ---------------- pools ----------------
            xw = ctx.enter_context(tc.tile_pool(name="xw", bufs=1))
            pp = ctx.enter_context(tc.tile_pool(name="pp", bufs=2, space="PSUM"))
            sp = ctx.enter_context(tc.tile_pool(name="sp", bufs=2, space="PSUM"))
            cp = ctx.enter_context(tc.tile_pool(name="cp", bufs=2, space="PSUM"))
            probs = ctx.enter_context(tc.tile_pool(name="probs", bufs=3))
            octp = ctx.enter_context(tc.tile_pool(name="octp", bufs=3))
            rcp = ctx.enter_context(tc.tile_pool(name="rcp", bufs=3))
            dupp = ctx.enter_context(tc.tile_pool(name="dupp", bufs=3))

            # consolidated loads: few large DMAs (HW splits each across the
            # 16 SDMA engines) instead of 32 small ones -- the per-DMA
            # dispatch on the sync sequencer (~0.6us each) was serializing
            # the whole startup.  Issue in consumption order; wv last.
            xta = xw.tile([PC, 8, S], BF16, tag="xta", name="xta")
            wqa = xw.tile([PC, 8, HG], BF16, tag="wqa", name="wqa")
            wka = xw.tile([PC, 8, HG], BF16, tag="wka", name="wka")
            wva = xw.tile([PC, 8, HG], BF16, tag="wva", name="wva")
            # head-pair-0 weight slices first so proj m0 (and with it the
            # first scores/exp) is gated on ~2.5MB of DMA, not 5MB
            nc.sync.dma_start(wqa[:, :, 0:PC],
                              WQT[:, 0:PC].rearrange("(k p) f -> p k f", p=PC))
            for q in range(4):
                nc.sync.dma_start(
                    xta[:, 2 * q:2 * q + 2, :],
                    XT[2 * q * PC:(2 * q + 2) * PC, :]
                    .rearrange("(k p) s -> p k s", p=PC))
            nc.sync.dma_start(wka[:, :, 0:PC],
                              WKT[:, 0:PC].rearrange("(k p) f -> p k f", p=PC))
            nc.sync.dma_start(wqa[:, :, PC:HG],
                              WQT[:, PC:HG].rearrange("(k p) f -> p k f", p=PC))
            nc.sync.dma_start(wka[:, :, PC:HG],
                              WKT[:, PC:HG].rearrange("(k p) f -> p k f", p=PC))
            nc.sync.dma_start(wva, WVT[:, :].rearrange("(k p) f -> p k f", p=PC))
            xts = [xta[:, k, :] for k in range(8)]
            wqs = [wqa[:, k, :] for k in range(8)]
            wks = [wka[:, k, :] for k in range(8)]
            wvs = [wva[:, k, :] for k in range(8)]

            def proj_qk(m):
                """Project head pair (2m, 2m+1) and build their stacked
                qaug/kaug tiles.  Psum half 0:64 is head 2m, 64:128 is
                head 2m+1; the other (scaled) half of each aug tile is a
                DMA partition-dup followed by an aligned DVE multiply."""
                h0, h1 = 2 * m, 2 * m + 1
                for wsrc, aug, brd, t in ((wqs, qaug, wm1b, "q"),
                                          (wks, kaug, mkb, "k")):
                    for n in range(2):
                        ps = pp.tile([PC, 512], F32, tag="ppsum",
                                     name=f"ppsum_{t}_{m}_{n}")
                        for k in range(8):
                            nc.tensor.matmul(
                                ps,
                                lhsT=wsrc[k][:, m * PC:(m + 1) * PC],
                                rhs=xts[k][:, n * 512:(n + 1) * 512],
                                start=(k == 0), stop=(k == 7),
                            )
                        qs = slice(n * 512, (n + 1) * 512)
                        nc.vector.tensor_copy(aug[0:HD, h0, qs], ps[0:HD, :])
                        nc.vector.tensor_copy(aug[HD:PC, h1, qs], ps[HD:PC, :])
                    dup = dupp.tile([PC, S], BF16, tag="dup",
                                    name=f"dup_{t}_{m}", bufs=3)
                    # SWDGE ring: keeps these latency-critical partition
                    # dups off the HWDGE ring that carries the big loads
                    nc.gpsimd.dma_start(dup[HD:PC, :], aug[0:HD, h0, :])
                    nc.gpsimd.dma_start(dup[0:HD, :], aug[HD:PC, h1, :])
                    nc.vector.tensor_mul(aug[HD:PC, h0, :], dup[HD:PC, :],
                                         brd[HD:PC, :])
                    nc.vector.tensor_mul(aug[0:HD, h1, :], dup[0:HD, :],
                                         brd[0:HD, :])

            def proj_v(half):
                """V s-chunks [4*half, 4*half+4)."""
                for sc in range(4 * half, 4 * half + 4):
                    ps = pp.tile([PC, 512], F32, tag="ppsum",
                                 name=f"vpsum_{sc}")
                    for k in range(8):
                        nc.tensor.matmul(
                            ps,
                            lhsT=xts[k][:, sc * PC:(sc + 1) * PC],
                            rhs=wvs[k][:, :],
                            start=(k == 0), stop=(k == 7),
                        )
                    nc.vector.tensor_copy(
                        vaug[:, sc, :, 0:HD],
                        ps.rearrange("p (h d) -> p h d", h=8),
                    )

            def act_reciprocal(out, in_):
                """Raw ACT Reciprocal (bypasses the bass-level ban; measured
                ~1e-5 rel err on HW - fine for softmax denominators, and it
                keeps the reciprocal off the DVE critical path)."""
                sc = nc.scalar
                ins = [sc.lower_ap(in_)]
                for v in (0.0, 1.0, 0.0):  # bias, scale, alpha
                    ins.append(mybir.ImmediateValue(dtype=mybir.dt.float32,
                                                    value=v))
                return sc.add_instruction(mybir.InstActivation(
                    name=nc.get_next_instruction_name(),
                    func=mybir.ActivationFunctionType.Reciprocal,
                    ins=ins, outs=[sc.lower_ap(out)]))

            def scores_head(h, pt):
                """scoresT + exp for one head -> fills pt[:, kc, :].
                One K=128 stacked matmul per (kc, qc) psum half."""
                for kc in range(8):
                    psc = sp.tile([PC, S], F32, tag="spsum",
                                  name=f"spsum_{h}_{kc}")
                    ks = slice(kc * PC, (kc + 1) * PC)
                    for qc in range(2):
                        qs = slice(qc * 512, (qc + 1) * 512)
                        nc.tensor.matmul(
                            psc[:, qs],
                            lhsT=kaug[:, h, ks],
                            rhs=qaug[:, h, qs],
                            start=True, stop=True,
                        )
                    nc.scalar.activation(
                        out=pt[:, kc, :], in_=psc[:, :],
                        func=Exp, scale=0.125,
                    )

            def ctx_head(h, pt):
                # accumulate ctx^T; evict psum fast (DVE copy of all 65
                # rows) so the PE never waits on the normalize chain.
                # Normalize is batched per head: one reciprocal, one
                # partition-broadcast, multiplies on GpSimd (DVE is the
                # hotter engine in the tail), one [64, S] store.
                css = []
                rc = rcp.tile([1, S], F32, tag="rc", name=f"rc_{h}")
                for qc in range(2):
                    qs = slice(qc * 512, (qc + 1) * 512)
                    cps = cp.tile([HD + 1, 512], F32, tag="cpsum",
                                  name=f"cpsum_{h}_{qc}")
                    for kc in range(8):
                        nc.tensor.matmul(
                            cps,
                            lhsT=vaug[:, kc, h, :],
                            rhs=pt[:, kc, qs],
                            start=(kc == 0), stop=(kc == 7),
                        )
                    cs = octp.tile([HD + 1, 512], F32, tag="cstage",
                                   name=f"cstage_{h}_{qc}", bufs=4)
                    nc.vector.tensor_copy(cs, cps[:, :])
                    nc.sync.dma_start(rc[:, qs], cs[HD:HD + 1, :])
                    css.append(cs)
                rc2 = rcp.tile([1, S], F32, tag="rc2", name=f"rc2_{h}")
                # approx reciprocal on DVE (~51 ULP, fine for softmax
                # denominators).  Exact `reciprocal()` costs 3.3us per
                # call; ACT Reciprocal lives in a different table set
                # than Exp and forces a 1.3us table reload per switch.
                # Denominators are sums of positive exps, so the approx
                # edge cases (0/denorm/inf) cannot occur.
                nc.vector.reciprocal_approx_fast(out=rc2[:, :], in_=rc[:, :])
                rb = rcp.tile([HD, S], F32, tag="rb", name=f"rb_{h}")
                nc.gpsimd.partition_broadcast(rb, rc2)
                ot = octp.tile([HD, S], F32, tag="ot", name=f"ot_{h}")
                for qc in range(2):
                    qs = slice(qc * 512, (qc + 1) * 512)
                    nc.gpsimd.tensor_mul(ot[:, qs], css[qc][0:HD, :],
                                         rb[:, qs])
                nc.sync.dma_start(OUT[h * HD:(h + 1) * HD, :], ot)

            def pthead(h):
                return probs.tile([PC, KC, S], BF16, tag="probs",
                                  name=f"probs_{h}", bufs=3)

            # Software pipeline at per-head granularity: scores(h) is
            # emitted as early as its aug tiles can be ready so ScalarE
            # starts draining exps ASAP (ACT is ~73us of exp work); proj
            # and the previous head's ctx fill the PE in between.
            proj_qk(0)
            pt0 = pthead(0); scores_head(0, pt0)
            proj_v(0)
            proj_qk(1)
            pt1 = pthead(1); scores_head(1, pt1)
            proj_v(1)
            ctx_head(0, pt0)
            proj_qk(2)
            pt2 = pthead(2); scores_head(2, pt2)
            ctx_head(1, pt1)
            proj_qk(3)
            pt3 = pthead(3); scores_head(3, pt3)
            ctx_head(2, pt2)
            pt4 = pthead(4); scores_head(4, pt4)
            ctx_head(3, pt3)
            pt5 = pthead(5); scores_head(5, pt5)
            ctx_head(4, pt4)
            pt6 = pthead(6); scores_head(6, pt6)
            ctx_head(5, pt5)
            pt7 = pthead(7); scores_head(7, pt7)
            ctx_head(6, pt6)
            ctx_head(7, pt7)

    nc.compile()
    return nc


_NC_CACHE = None


def _get_program():
    global _NC_CACHE
    if _NC_CACHE is None:
        _NC_CACHE = _build_program()
    return _NC_CACHE


def kernel(hidden_states, attention_mask, sep_idx, Wq, bq, Wk, bk, Wv, bv,
           w0, w1):
    hs = np.asarray(hidden_states, dtype=np.float32)
    Wq = np.asarray(Wq, dtype=np.float32)
    Wk = np.asarray(Wk, dtype=np.float32)
    Wv = np.asarray(Wv, dtype=np.float32)
    sep = np.asarray(sep_idx)
    w0c = float(np.clip(np.asarray(w0, np.float32)[0], 0.0, 0.5))
    w1c = float(np.clip(np.asarray(w1, np.float32)[0], 0.5, 1.0))
    idx2 = np.asarray(sep[:, 2], dtype=np.int64)

    bf = ml_dtypes.bfloat16
    pos = np.arange(S)

    # per-batch host-side shard prep (layout transforms only)
    xt_b = [np.ascontiguousarray(hs[b].T).astype(bf) for b in range(B)]
    wm1_b = []
    mk_b = []
    for b in range(B):
        wseg = np.where(pos < idx2[b], w0c, w1c).astype(np.float32) - 1.0
        wm1_b.append(wseg.reshape(1, S).astype(bf))
        mk_b.append((pos >= idx2[b]).astype(np.float32).reshape(1, S).astype(bf))
    wqt_g = [np.ascontiguousarray(Wq[g * HG:(g + 1) * HG, :].T).astype(bf)
             for g in range(2)]
    wkt_g = [np.ascontiguousarray(Wk[g * HG:(g + 1) * HG, :].T).astype(bf)
             for g in range(2)]
    wvt_g = [np.ascontiguousarray(Wv[g * HG:(g + 1) * HG, :].T).astype(bf)
             for g in range(2)]

    in_maps = []
    for c in range(NCORES):
        b, g = c % B, c // B
        in_maps.append({
            "xt": xt_b[b],
            "wqt": wqt_g[g],
            "wkt": wkt_g[g],
            "wvt": wvt_g[g],
            "wm1": wm1_b[b],
            "mkey": mk_b[b],
        })

    nc = _get_program()
    res = run_bass_kernel_spmd(nc, in_maps, core_ids=list(range(NCORES)))

    out = np.empty((B, S, H), dtype=np.float32)
    for c in range(NCORES):
        b, g = c % B, c // B
        out[b, :, g * HG:(g + 1) * HG] = res.results[c]["out_t"].T
    return out


# revision 14
# speedup vs baseline: 1.8082x; 1.8082x over previous
"""Trainium2 Bass kernel for nn_BertSelfAttention_43404939493966.

BERT self-attention with adaptive per-segment scaling:
  q/k/v = hidden @ W{q,k,v}.T + b        (biases are spec'd zero -> skipped)
  scores = q k^T / 8,  scaled per (batch,row,col) segment rule, softmax, @v

Sharding: 8 cores = 4 batches x 2 head-groups (8 heads each).
Each core gets host-pretransposed bf16 operands:
  xt  = hidden[b].T            [H=1024, S=1024]
  w?t = W[g*512:(g+1)*512].T   [1024, 512]
  wm1 = (w_seg(q) - 1)         [1, S]   (w_seg = w0c if q < idx2 else w1c)
  mkey= 1[key >= idx2]         [1, S]
and returns ctx^T for its head-group  [512, S] f32.

Device algorithm (per core, one SPMD program):
  Segment scaling is exact via scale(k,q) = 1 + mkey(k)*(w(q)-1):
    scoresT = KT^T.QT + (KT*mkey)^T.(QT*(w-1))
  Both terms are computed in a SINGLE full-width (K=128) matmul by
  stacking per head h the pair [k_h ; k_h*mkey] (kaug) against
  [q_h ; q_h*(w-1)] (qaug) on the partition axis.  The stacked halves
  are built from the projection psums with partition-aligned DVE
  copies, a partition-shifted SBUF->SBUF DMA duplicate, and an aligned
  DVE multiply (even heads: raw top/scaled bottom; odd heads reversed,
  matching the psum half each head lands in).
  exp on ScalarE (scale=1/8 folded into the activation), output bf16.
  ctx^T = V_aug^T @ probsT with V augmented by a ones-column, so the
  softmax denominator falls out of the same matmul (psum row 64);
  normalize with reciprocal + partition-broadcast + multiply.

attention_mask is all-zeros by spec (fill=zeros) and is not applied.
"""

import numpy as np
import ml_dtypes
from contextlib import ExitStack

import concourse.bass as bass
import concourse.tile as tile
from concourse import bacc, mybir
from concourse.bass_utils import run_bass_kernel_spmd

B, S, H = 4, 1024, 1024
NH, HD = 16, 64
NCORES = 8
HG = 512          # head-group width (8 heads x 64)
KC = 8            # 128-wide key chunks
PC = 128

BF16 = mybir.dt.bfloat16
F32 = mybir.dt.float32


def _build_program():
    nc = bacc.Bacc("TRN2", target_bir_lowering=False, debug=False)

    XT = nc.dram_tensor("xt", (H, S), BF16, kind="ExternalInput")
    WQT = nc.dram_tensor("wqt", (H, HG), BF16, kind="ExternalInput")
    WKT = nc.dram_tensor("wkt", (H, HG), BF16, kind="ExternalInput")
    WVT = nc.dram_tensor("wvt", (H, HG), BF16, kind="ExternalInput")
    WM1 = nc.dram_tensor("wm1", (1, S), BF16, kind="ExternalInput")
    MKEY = nc.dram_tensor("mkey", (1, S), BF16, kind="ExternalInput")
    OUT = nc.dram_tensor("out_t", (HG, S), F32, kind="ExternalOutput")

    Exp = mybir.ActivationFunctionType.Exp

    with tile.TileContext(nc) as tc:
        with ExitStack() as ctx:
            persist = ctx.enter_context(tc.tile_pool(name="persist", bufs=1))

            # stacked score operands: [:, h, :] is head h's 128-deep
            # contraction tile ([raw;scaled] even h, [scaled;raw] odd h)
            qaug = persist.tile([PC, 8, S], BF16)
            kaug = persist.tile([PC, 8, S], BF16)
            vaug = persist.tile([PC, 8, 8, HD + 1], BF16)  # [p, s-chunk, head, d+1]
            wm1b = persist.tile([PC, S], BF16)
            mkb = persist.tile([PC, S], BF16)

            # load the per-q / per-key vectors ([1,S] rows), broadcast on
            # GpSimd (keeps the startup DMA path free for the big loads)
            wrow = persist.tile([1, S], BF16)
            mrow = persist.tile([1, S], BF16)
            nc.sync.dma_start(wrow, WM1[:, :])
            nc.sync.dma_start(mrow, MKEY[:, :])
            nc.gpsimd.partition_broadcast(wm1b, wrow)
            nc.gpsimd.partition_broadcast(mkb, mrow)
            nc.vector.memset(vaug[:, :, :, HD:HD + 1], 1.0)

            # ---------------- pools ----------------
            xw = ctx.enter_context(tc.tile_pool(name="xw", bufs=1))
            pp = ctx.enter_context(tc.tile_pool(name="pp", bufs=2, space="PSUM"))
            sp = ctx.enter_context(tc.tile_pool(name="sp", bufs=2, space="PSUM"))
            cp = ctx.enter_context(tc.tile_pool(name="cp", bufs=2, space="PSUM"))
            probs = ctx.enter_context(tc.tile_pool(name="probs", bufs=3))
            octp = ctx.enter_context(tc.tile_pool(name="octp", bufs=3))
            rcp = ctx.enter_context(tc.tile_pool(name="rcp", bufs=3))
            dupp = ctx.enter_context(tc.tile_pool(name="dupp", bufs=3))

            # consolidated loads: few large DMAs (HW splits each across the
            # 16 SDMA engines) instead of 32 small ones -- the per-DMA
            # dispatch on the sync sequencer (~0.6us each) was serializing
            # the whole startup.  Issue in consumption order; wv last.
            xta = xw.tile([PC, 8, S], BF16, tag="xta", name="xta")
            wqa = xw.tile([PC, 8, HG], BF16, tag="wqa", name="wqa")
            wka = xw.tile([PC, 8, HG], BF16, tag="wka", name="wka")
            wva = xw.tile([PC, 8, HG], BF16, tag="wva", name="wva")
            nc.sync.dma_start(wqa, WQT[:, :].rearrange("(k p) f -> p k f", p=PC))
            nc.sync.dma_start(xta[:, 0:4, :],
                              XT[0:4 * PC, :].rearrange("(k p) s -> p k s", p=PC))
            nc.sync.dma_start(xta[:, 4:8, :],
                              XT[4 * PC:8 * PC, :].rearrange("(k p) s -> p k s", p=PC))
            nc.sync.dma_start(wka, WKT[:, :].rearrange("(k p) f -> p k f", p=PC))
            nc.sync.dma_start(wva, WVT[:, :].rearrange("(k p) f -> p k f", p=PC))
            xts = [xta[:, k, :] for k in range(8)]
            wqs = [wqa[:, k, :] for k in range(8)]
            wks = [wka[:, k, :] for k in range(8)]
            wvs = [wva[:, k, :] for k in range(8)]

            def proj_qk(m):
                """Project head pair (2m, 2m+1) and build their stacked
                qaug/kaug tiles.  Psum half 0:64 is head 2m, 64:128 is
                head 2m+1; the other (scaled) half of each aug tile is a
                DMA partition-dup followed by an aligned DVE multiply."""
                h0, h1 = 2 * m, 2 * m + 1
                for wsrc, aug, brd, t in ((wqs, qaug, wm1b, "q"),
                                          (wks, kaug, mkb, "k")):
                    for n in range(2):
                        ps = pp.tile([PC, 512], F32, tag="ppsum",
                                     name=f"ppsum_{t}_{m}_{n}")
                        for k in range(8):
                            nc.tensor.matmul(
                                ps,
                                lhsT=wsrc[k][:, m * PC:(m + 1) * PC],
                                rhs=xts[k][:, n * 512:(n + 1) * 512],
                                start=(k == 0), stop=(k == 7),
                            )
                        qs = slice(n * 512, (n + 1) * 512)
                        nc.vector.tensor_copy(aug[0:HD, h0, qs], ps[0:HD, :])
                        nc.vector.tensor_copy(aug[HD:PC, h1, qs], ps[HD:PC, :])
                    dup = dupp.tile([PC, S], BF16, tag="dup",
                                    name=f"dup_{t}_{m}", bufs=3)
                    # SWDGE ring: keeps these latency-critical partition
                    # dups off the HWDGE ring that carries the big loads
                    nc.gpsimd.dma_start(dup[HD:PC, :], aug[0:HD, h0, :])
                    nc.gpsimd.dma_start(dup[0:HD, :], aug[HD:PC, h1, :])
                    nc.vector.tensor_mul(aug[HD:PC, h0, :], dup[HD:PC, :],
                                         brd[HD:PC, :])
                    nc.vector.tensor_mul(aug[0:HD, h1, :], dup[0:HD, :],
                                         brd[0:HD, :])

            def proj_v(half):
                """V s-chunks [4*half, 4*half+4)."""
                for sc in range(4 * half, 4 * half + 4):
                    ps = pp.tile([PC, 512], F32, tag="ppsum",
                                 name=f"vpsum_{sc}")
                    for k in range(8):
                        nc.tensor.matmul(
                            ps,
                            lhsT=xts[k][:, sc * PC:(sc + 1) * PC],
                            rhs=wvs[k][:, :],
                            start=(k == 0), stop=(k == 7),
                        )
                    nc.vector.tensor_copy(
                        vaug[:, sc, :, 0:HD],
                        ps.rearrange("p (h d) -> p h d", h=8),
                    )

            def act_reciprocal(out, in_):
                """Raw ACT Reciprocal (bypasses the bass-level ban; measured
                ~1e-5 rel err on HW - fine for softmax denominators, and it
                keeps the reciprocal off the DVE critical path)."""
                sc = nc.scalar
                ins = [sc.lower_ap(in_)]
                for v in (0.0, 1.0, 0.0):  # bias, scale, alpha
                    ins.append(mybir.ImmediateValue(dtype=mybir.dt.float32,
                                                    value=v))
                return sc.add_instruction(mybir.InstActivation(
                    name=nc.get_next_instruction_name(),
                    func=mybir.ActivationFunctionType.Reciprocal,
                    ins=ins, outs=[sc.lower_ap(out)]))

            def scores_head(h, pt):
                """scoresT + exp for one head -> fills pt[:, kc, :].
                One K=128 stacked matmul per (kc, qc) psum half."""
                for kc in range(8):
                    psc = sp.tile([PC, S], F32, tag="spsum",
                                  name=f"spsum_{h}_{kc}")
                    ks = slice(kc * PC, (kc + 1) * PC)
                    for qc in range(2):
                        qs = slice(qc * 512, (qc + 1) * 512)
                        nc.tensor.matmul(
                            psc[:, qs],
                            lhsT=kaug[:, h, ks],
                            rhs=qaug[:, h, qs],
                            start=True, stop=True,
                        )
                    nc.scalar.activation(
                        out=pt[:, kc, :], in_=psc[:, :],
                        func=Exp, scale=0.125,
                    )

            def ctx_mm(h, pt):
                """ctx^T matmuls + fast psum eviction + denominator
                extraction for head h.  Returns the staging tiles; the
                normalize finish is a separate emission (ctx_fin) lagged
                one head behind, so these PE-blocking psum evictions are
                always ahead of older normalize work in the DVE queue."""
                parts = []
                for qc in range(2):
                    qs = slice(qc * 512, (qc + 1) * 512)
                    cps = cp.tile([HD + 1, 512], F32, tag="cpsum",
                                  name=f"cpsum_{h}_{qc}")
                    for kc in range(8):
                        nc.tensor.matmul(
                            cps,
                            lhsT=vaug[:, kc, h, :],
                            rhs=pt[:, kc, qs],
                            start=(kc == 0), stop=(kc == 7),
                        )
                    cs = octp.tile([HD + 1, 512], F32, tag="cstage",
                                   name=f"cstage_{h}_{qc}", bufs=4)
                    nc.vector.tensor_copy(cs, cps[:, :])
                    rc = rcp.tile([1, 512], F32, tag="rc",
                                  name=f"rc_{h}_{qc}", bufs=4)
                    nc.sync.dma_start(rc[:, :], cs[HD:HD + 1, :])
                    parts.append((cs, rc))
                return parts

            def ctx_fin(h, parts):
                for qc, (cs, rc) in enumerate(parts):
                    qs = slice(qc * 512, (qc + 1) * 512)
                    rc2 = rcp.tile([1, 512], F32, tag="rc2",
                                   name=f"rc2_{h}_{qc}")
                    # approx reciprocal on DVE (~51 ULP, fine for softmax
                    # denominators).  Exact `reciprocal()` costs 3.3us per
                    # call; ACT Reciprocal lives in a different table set
                    # than Exp and forces a 1.3us table reload per switch.
                    # Denominators are sums of positive exps, so the
                    # approx edge cases (0/denorm/inf) cannot occur.
                    nc.vector.reciprocal_approx_fast(out=rc2[:, :],
                                                     in_=rc[:, :])
                    rb = rcp.tile([HD, 512], F32, tag="rb",
                                  name=f"rb_{h}_{qc}")
                    nc.gpsimd.partition_broadcast(rb, rc2)
                    ot = octp.tile([HD, 512], F32, tag="ot",
                                   name=f"ot_{h}_{qc}")
                    nc.vector.tensor_mul(ot, cs[0:HD, :], rb)
                    nc.sync.dma_start(OUT[h * HD:(h + 1) * HD, qs], ot)

            def ctx_head(h, pt):
                ctx_fin(h, ctx_mm(h, pt))

            def pthead(h):
                return probs.tile([PC, KC, S], BF16, tag="probs",
                                  name=f"probs_{h}", bufs=3)

            # Software pipeline at per-head granularity: scores(h) is
            # emitted as early as its aug tiles can be ready so ScalarE
            # starts draining exps ASAP (ACT is ~73us of exp work); proj
            # and the previous head's ctx fill the PE in between.
            proj_qk(0)
            pt0 = pthead(0); scores_head(0, pt0)
            proj_v(0)
            proj_qk(1)
            pt1 = pthead(1); scores_head(1, pt1)
            proj_v(1)
            cx0 = ctx_mm(0, pt0)
            proj_qk(2)
            pt2 = pthead(2); scores_head(2, pt2)
            cx1 = ctx_mm(1, pt1)
            ctx_fin(0, cx0)
            proj_qk(3)
            pt3 = pthead(3); scores_head(3, pt3)
            cx2 = ctx_mm(2, pt2)
            ctx_fin(1, cx1)
            pt4 = pthead(4); scores_head(4, pt4)
            cx3 = ctx_mm(3, pt3)
            ctx_fin(2, cx2)
            pt5 = pthead(5); scores_head(5, pt5)
            cx4 = ctx_mm(4, pt4)
            ctx_fin(3, cx3)
            pt6 = pthead(6); scores_head(6, pt6)
            cx5 = ctx_mm(5, pt5)
            ctx_fin(4, cx4)
            pt7 = pthead(7); scores_head(7, pt7)
            cx6 = ctx_mm(6, pt6)
            ctx_fin(5, cx5)
            cx7 = ctx_mm(7, pt7)
            ctx_fin(6, cx6)
            ctx_fin(7, cx7)

    nc.compile()
    return nc


_NC_CACHE = None


def _get_program():
    global _NC_CACHE
    if _NC_CACHE is None:
        _NC_CACHE = _build_program()
    return _NC_CACHE


def kernel(hidden_states, attention_mask, sep_idx, Wq, bq, Wk, bk, Wv, bv,
           w0, w1):
    hs = np.asarray(hidden_states, dtype=np.float32)
    Wq = np.asarray(Wq, dtype=np.float32)
    Wk = np.asarray(Wk, dtype=np.float32)
    Wv = np.asarray(Wv, dtype=np.float32)
    sep = np.asarray(sep_idx)
    w0c = float(np.clip(np.asarray(w0, np.float32)[0], 0.0, 0.5))
    w1c = float(np.clip(np.asarray(w1, np.float32)[0], 0.5, 1.0))
    idx2 = np.asarray(sep[:, 2], dtype=np.int64)

    bf = ml_dtypes.bfloat16
    pos = np.arange(S)

    # per-batch host-side shard prep (layout transforms only)
    xt_b = [np.ascontiguousarray(hs[b].T).astype(bf) for b in range(B)]
    wm1_b = []
    mk_b = []
    for b in range(B):
        wseg = np.where(pos < idx2[b], w0c, w1c).astype(np.float32) - 1.0
        wm1_b.append(wseg.reshape(1, S).astype(bf))
        mk_b.append((pos >= idx2[b]).astype(np.float32).reshape(1, S).astype(bf))
    wqt_g = [np.ascontiguousarray(Wq[g * HG:(g + 1) * HG, :].T).astype(bf)
             for g in range(2)]
    wkt_g = [np.ascontiguousarray(Wk[g * HG:(g + 1) * HG, :].T).astype(bf)
             for g in range(2)]
    wvt_g = [np.ascontiguousarray(Wv[g * HG:(g + 1) * HG, :].T).astype(bf)
             for g in range(2)]

    in_maps = []
    for c in range(NCORES):
        b, g = c % B, c // B
        in_maps.append({
            "xt": xt_b[b],
            "wqt": wqt_g[g],
            "wkt": wkt_g[g],
            "wvt": wvt_g[g],
            "wm1": wm1_b[b],
            "mkey": mk_b[b],
        })

    nc = _get_program()
    res = run_bass_kernel_spmd(nc, in_maps, core_ids=list(range(NCORES)))

    out = np.empty((B, S, H), dtype=np.float32)
    for c in range(NCORES):
        b, g = c % B, c // B
        out[b, :, g * HG:(g + 1) * HG] = res.results[c]["out_t"].T
    return out
